# revision 1
# baseline (speedup 1.0000x reference)
"""Trainium2 Bass kernel for nn_EquiConv (e3nn-style FullyConnectedTensorProduct
+ gate + radial-MLP elementwise conv), data-parallel over edges on 8 cores.

Per core, 256-edge supertiles of two 128-edge subtiles, software-pipelined
three rounds deep (A: loads/krons/step1, B: mults + chunk/reduce matmuls,
C: gate + output tail), with the radial-MLP legs staggered across rounds:
  - ss/vv paths: DVE (+GPSIMD) build per-edge outer-product krons edge-major,
    one DMA-transpose per subtile flips all 40 k-chunks into a k-major SBUF
    supertile, PE accumulates 40 chunk matmuls into PSUM.
  - sv/vs paths: factorized. PE contracts the 64-wide scalar side first
    (weights stationary, x1sT/x2sT moving, per subtile into PSUM), ACT evicts
    to shared bf16 supertiles, DVE/GPSIMD multiply by host-replicated x2v/x1v
    "stack" operands, PE selector-matmuls reduce over v and scatter slice
    rows into the vec PSUM feature-major.
  - ACT uses sigmoid-only activation tables (silu decomposed as x*sigmoid(x)
    with the multiply on GPSIMD) to avoid per-supertile table reloads.
  - Gate + elementwise conv fused feature-major; bf16 outputs, host
    reassembles/transposes.
"""

import sys

sys.path.insert(0, "/opt/trn_rl_repo")

import numpy as np
import ml_dtypes

import concourse.bass as bass
import concourse.bacc as bacc
import concourse.mybir as mybir
import concourse.tile as tile
from concourse.bass_utils import run_bass_kernel_spmd

BF16 = ml_dtypes.bfloat16

E = 20000
S = 64
V = 32
FC_IN = 128
HID = 64
INV_SQRT3 = 0.5773502691896258

NCORES = 8
EC = E // NCORES  # 2500 edges per core
ET = 128  # edges per subtile
NT = (EC + ET - 1) // ET  # 20 subtiles
EPAD = NT * ET  # 2560
NSUB = 2
EW = NSUB * ET  # 256 edges per supertile
NSUP = EPAD // EW  # 10

A_SC = float(1.0 / np.sqrt(np.float32(S * S + V * V)))
A_VEC = float(1.0 / np.sqrt(np.float32(2 * S * V)))

f32 = mybir.dt.float32
bf16 = mybir.dt.bfloat16

N_SS = (S * S) // 128  # 32 ss chunks
N_VV = (V * V) // 128  # 8 vv chunks (i-summed)
NCH = N_SS + N_VV      # 40 chunks -> 96-wide out (sc|g)
NSL = 8                # (dw,v) slices per sv/vs step-1 (8 x 128 rows)

# packed-constant column offsets (bf16 [128, WCONST])
OFF_WSSVV = 0
OFF_WSV = OFF_WSSVV + NCH * (S + V)
OFF_WVS = OFF_WSV + NSL * 4 * V
OFF_SEL = OFF_WVS + NSL * 4 * V
OFF_FC1 = OFF_SEL + NSL * V
OFF_FC2 = OFF_FC1 + HID
OFF_FC3 = OFF_FC2 + HID
OFF_SEL3 = OFF_FC3 + S + V
WCONST = OFF_SEL3 + 3 * V


def _prep_weights(w_ss_s, w_vv_s, w_ss_g, w_vv_g, w_sv_v, w_vs_v,
                  fc_w1, fc_b1, fc_w2, fc_b2, fc_w3, fc_b3):
    """Host-side rearrangement of the shared weights."""
    wss = np.concatenate([w_ss_s, w_ss_g], axis=2) * A_SC  # [64,64,96]
    wvv = np.concatenate([w_vv_s, w_vv_g], axis=2) * (A_SC * INV_SQRT3)  # [32,32,96]
    w_ssvv = np.concatenate(
        [wss.reshape(S * S, S + V), wvv.reshape(V * V, S + V)], axis=0
    )  # [5120, 96];  k = u*64+v (ss) ++ 4096 + u*32+v (vv)
    w_ssvv = (
        w_ssvv.reshape(NCH, 128, S + V).transpose(1, 0, 2)
        .reshape(128, NCH * (S + V))
    )

    # sv step1 stationary: [u, (s,dw,v)] = w_sv_v[u, v, s*4+dw] * A_VEC
    wsv_mat = (w_sv_v * A_VEC).transpose(0, 2, 1).reshape(S, NSL * 4 * V)
    # vs step1 stationary: [vs, (s,dw,uv)] = w_vs_v[uv, vs, s*4+dw] * A_VEC
    wvs_mat = (w_vs_v * A_VEC).transpose(1, 2, 0).reshape(S, NSL * 4 * V)

    # selector for the v-reduce: sel[p=(dw,v), s, w'] = 1 iff w' == s*4 + p//32
    sel = np.zeros((128, NSL, V), dtype=np.float32)
    for p in range(128):
        dw = p // 32
        for s in range(NSL):
            sel[p, s, s * 4 + dw] = 1.0

    sel3 = np.zeros((V, 3 * V), dtype=np.float32)  # replicate [32] -> [(i,w)=96]
    for i in range(3):
        for w in range(V):
            sel3[w, i * V + w] = 1.0

    wpack = np.zeros((128, WCONST), BF16)
    wpack[:, OFF_WSSVV:OFF_WSSVV + NCH * (S + V)] = w_ssvv.astype(BF16)
    wpack[0:S, OFF_WSV:OFF_WSV + NSL * 4 * V] = wsv_mat.astype(BF16)
    wpack[0:S, OFF_WVS:OFF_WVS + NSL * 4 * V] = wvs_mat.astype(BF16)
    wpack[:, OFF_SEL:OFF_SEL + NSL * V] = sel.reshape(128, NSL * V).astype(BF16)
    wpack[0:FC_IN, OFF_FC1:OFF_FC1 + HID] = fc_w1.astype(BF16)
    wpack[0:HID, OFF_FC2:OFF_FC2 + HID] = fc_w2.astype(BF16)
    wpack[0:HID, OFF_FC3:OFF_FC3 + S + V] = fc_w3.astype(BF16)
    wpack[0:V, OFF_SEL3:OFF_SEL3 + 3 * V] = sel3.astype(BF16)
    bpack = np.zeros((S + V, 3), np.float32)
    bpack[0:HID, 0] = fc_b1
    bpack[0:HID, 1] = fc_b2
    bpack[:, 2] = fc_b3
    return {"wpack": wpack, "bpack": bpack}


def _build_program():
    nc = bacc.Bacc("TRN2", target_bir_lowering=False, debug=False)

    d_fea = nc.dram_tensor("fea", [EPAD, 320], bf16, kind="ExternalInput").ap()
    d_feaT = nc.dram_tensor("feaT", [NSUP, S, 2, EW], bf16, kind="ExternalInput").ap()
    d_stk = nc.dram_tensor("stk", [NSUP, 128, 6, EW], bf16, kind="ExternalInput").ap()
    d_fwT = nc.dram_tensor("fwT", [FC_IN, EPAD], bf16, kind="ExternalInput").ap()
    d_wpack = nc.dram_tensor("wpack", [128, WCONST], bf16, kind="ExternalInput").ap()
    d_bpack = nc.dram_tensor("bpack", [S + V, 3], f32, kind="ExternalInput").ap()

    d_osc = nc.dram_tensor("out_sc", [S, EPAD], bf16, kind="ExternalOutput").ap()
    d_ovec = nc.dram_tensor("out_vec", [3 * V, EPAD], bf16, kind="ExternalOutput").ap()

    SiLU = mybir.ActivationFunctionType.Silu
    Sigm = mybir.ActivationFunctionType.Sigmoid
    Copy = mybir.ActivationFunctionType.Copy
    Ident = mybir.ActivationFunctionType.Identity
    mul_op = mybir.AluOpType.mult
    add_op = mybir.AluOpType.add

    with tile.TileContext(nc) as tc:
        with (
            tc.tile_pool(name="consts", bufs=1) as consts,
            tc.tile_pool(name="io", bufs=3) as io,
            tc.tile_pool(name="kron", bufs=2) as kronp,
            tc.tile_pool(name="ktr", bufs=2) as ktrp,
            tc.tile_pool(name="tsb", bufs=2) as tsbp,
            tc.tile_pool(name="mm", bufs=2) as mmp,
            tc.tile_pool(name="post", bufs=4) as postp,
            tc.tile_pool(name="pT", bufs=3, space=bass.MemorySpace.PSUM) as pT,
            tc.tile_pool(name="pacc", bufs=3, space=bass.MemorySpace.PSUM) as pacc,
            tc.tile_pool(name="pmlp", bufs=2, space=bass.MemorySpace.PSUM) as pmlp,
        ):
            # ---- constants (resident, one packed bf16 DMA + one f32 DMA) ----
            wpack = consts.tile([128, WCONST], bf16, name="wpack")
            nc.sync.dma_start(wpack[:], d_wpack)
            wssvv = wpack[:, OFF_WSSVV:OFF_WSSVV + NCH * (S + V)]
            wsv = wpack[0:S, OFF_WSV:OFF_WSV + NSL * 4 * V]
            wvs = wpack[0:S, OFF_WVS:OFF_WVS + NSL * 4 * V]
            sel = wpack[:, OFF_SEL:OFF_SEL + NSL * V]
            wfc1 = wpack[0:FC_IN, OFF_FC1:OFF_FC1 + HID]
            wfc2 = wpack[0:HID, OFF_FC2:OFF_FC2 + HID]
            wfc3 = wpack[0:HID, OFF_FC3:OFF_FC3 + S + V]
            sel3 = wpack[0:V, OFF_SEL3:OFF_SEL3 + 3 * V]
            bpack = consts.tile([S + V, 3], f32, name="bpack")
            nc.sync.dma_start(bpack[:], d_bpack)
            bfc1 = bpack[0:HID, 0:1]
            bfc2 = bpack[0:HID, 1:2]
            bfc3 = bpack[0:S + V, 2:3]

            state = {}

            def load_fwT(sp):
                fwT = io.tile([FC_IN, EW], bf16, tag="fwT", name=f"fwT_{sp}")
                nc.sync.dma_start(fwT[:], d_fwT[:, sp * EW:sp * EW + EW])
                state.setdefault(sp, {})["fwT"] = fwT

            def loads(sp):
                st = state.setdefault(sp, {})
                g0 = sp * EW
                feaT = io.tile([S, 2, EW], bf16, tag="feaT", name=f"feaT_{sp}")
                nc.sync.dma_start(feaT[:], d_feaT[sp])
                stk = io.tile([128, 6, EW], bf16, tag="stk", name=f"stk_{sp}")
                nc.sync.dma_start(stk[:], d_stk[sp])
                st["feaT"], st["stk"] = feaT, stk
                st["fea"] = []
                for s in range(NSUB):
                    e0 = g0 + s * ET
                    fea = io.tile([ET, 320], bf16, tag=f"fea_{s}", name=f"fea_{sp}_{s}")
                    nc.sync.dma_start(fea[:], d_fea[e0:e0 + ET, :])
                    st["fea"].append(fea)

            def mlp1(sp):
                st = state[sp]
                h1p = pmlp.tile([S + V, EW], f32, tag="mlp", name=f"h1p_{sp}")
                nc.tensor.matmul(h1p[0:HID, :], wfc1, st["fwT"][:], start=True, stop=True)
                h1b = postp.tile([HID, EW], bf16, tag="h1b", name=f"h1b_{sp}")
                nc.scalar.activation(h1b[:], h1p[0:HID, :], Ident, bias=bfc1)
                h1g = postp.tile([HID, EW], bf16, tag="h1g", name=f"h1g_{sp}")
                nc.scalar.activation(h1g[:], h1p[0:HID, :], Sigm, bias=bfc1)
                st["h1parts"] = (h1b, h1g)

            def mlp2(sp):
                st = state[sp]
                h2p = pmlp.tile([S + V, EW], f32, tag="mlp", name=f"h2p_{sp}")
                nc.tensor.matmul(h2p[0:HID, :], wfc2, st["h1"][:], start=True, stop=True)
                h2b = postp.tile([HID, EW], bf16, tag="h2b", name=f"h2b_{sp}")
                nc.scalar.activation(h2b[:], h2p[0:HID, :], Ident, bias=bfc2)
                h2g = postp.tile([HID, EW], bf16, tag="h2g", name=f"h2g_{sp}")
                nc.scalar.activation(h2g[:], h2p[0:HID, :], Sigm, bias=bfc2)
                st["h2parts"] = (h2b, h2g)

            def mlp1g(sp):
                st = state[sp]
                h1b, h1g = st["h1parts"]
                h1 = postp.tile([HID, EW], bf16, tag="h1", name=f"h1_{sp}")
                nc.gpsimd.tensor_tensor(h1[:], h1b[:], h1g[:], mul_op)
                st["h1"] = h1

            def mlp2g(sp):
                st = state[sp]
                h2b, h2g = st["h2parts"]
                h2 = postp.tile([HID, EW], bf16, tag="h2", name=f"h2_{sp}")
                nc.gpsimd.tensor_tensor(h2[:], h2b[:], h2g[:], mul_op)
                st["h2"] = h2

            def mlp3(sp):
                st = state[sp]
                wp = pmlp.tile([S + V, EW], f32, tag="mlp", name=f"wp_{sp}")
                nc.tensor.matmul(wp[:], wfc3, st["h2"][:], start=True, stop=True)
                wgt_sc = postp.tile([S, EW], bf16, tag="wgt_sc", name=f"wgt_sc_{sp}")
                nc.scalar.activation(wgt_sc[:], wp[0:S, :], Ident, bias=bfc3[0:S, :])
                wgt_v = postp.tile([V, EW], bf16, tag="wgt_v", name=f"wgt_v_{sp}")
                nc.scalar.activation(wgt_v[:], wp[S:S + V, :], Ident, bias=bfc3[S:S + V, :])
                st["wgt_sc"], st["wgt_v"] = wgt_sc, wgt_v

            def prebuilds(sp):
                st = state[sp]
                st["pre"] = []
                for s in range(NSUB):
                    fea = st["fea"][s]
                    x1s2 = io.tile([ET, 2 * S], bf16, tag=f"x1s2_{s}", name=f"x1s2_{sp}_{s}")
                    nc.scalar.activation(
                        x1s2[:].rearrange("e (u p) -> e u p", p=2),
                        fea[:, 0:S].unsqueeze(2).broadcast_to([ET, S, 2]), Copy)
                    x2vg = io.tile([ET, 3 * V], bf16, tag=f"x2vg_{s}", name=f"x2vg_{sp}_{s}")
                    nc.scalar.activation(
                        x2vg[:].rearrange("e (i u) -> e i u", u=V),
                        fea[:, 160 + S:320].rearrange("e (u i) -> e i u", i=3), Copy)
                    x1vg2 = io.tile([ET, 6 * V], bf16, tag=f"x1vg2_{s}", name=f"x1vg2_{sp}_{s}")
                    nc.scalar.activation(
                        x1vg2[:].rearrange("e (i u p) -> e i u p", u=V, p=2),
                        fea[:, S:160].rearrange("e (u i) -> e i u", i=3)
                            .unsqueeze(3).broadcast_to([ET, 3, V, 2]), Copy)
                    st["pre"].append((x1s2, x2vg, x1vg2))

            def krons(sp, s):
                st = state[sp]
                fea = st["fea"][s]
                x1s2, x2vg, x1vg2 = st["pre"][s]
                x2s = fea[:, 160:160 + S]
                if s == 0:
                    st["st_k"] = ktrp.tile([128, NCH, EW], bf16, tag="st_k", name=f"st_k_{sp}")
                kron = kronp.tile([ET, 5120], bf16, tag="kron", name=f"kron_{sp}_{s}")
                nc.vector.tensor_tensor(
                    kron[:, 0:S * S].rearrange("e (u vh p) -> e u vh p", vh=S // 2, p=2),
                    x1s2[:].rearrange("e (u p) -> e u p", p=2)
                        .unsqueeze(2).broadcast_to([ET, S, S // 2, 2]),
                    x2s.rearrange("e (vh p) -> e vh p", p=2)
                        .unsqueeze(1).broadcast_to([ET, S, S // 2, 2]),
                    mul_op)
                pv = [kronp.tile([ET, V * V], bf16, tag=f"pv{i}", name=f"pv{i}_{sp}_{s}") for i in range(2)]
                for i in range(3):
                    dst = (kron[:, S * S:S * S + V * V] if i == 2 else pv[i][:])
                    eng = nc.vector
                    eng.tensor_tensor(
                        dst.rearrange("e (u vh p) -> e u vh p", vh=V // 2, p=2),
                        x1vg2[:, i * 2 * V:(i + 1) * 2 * V]
                            .rearrange("e (u p) -> e u p", p=2)
                            .unsqueeze(2).broadcast_to([ET, V, V // 2, 2]),
                        x2vg[:, i * V:(i + 1) * V]
                            .rearrange("e (vh p) -> e vh p", p=2)
                            .unsqueeze(1).broadcast_to([ET, V, V // 2, 2]),
                        mul_op)
                kvv = kron[:, S * S:S * S + V * V]
                nc.vector.tensor_tensor(kvv, kvv, pv[0][:], add_op)
                nc.vector.tensor_tensor(kvv, kvv, pv[1][:], add_op)
                nc.sync.dma_start(st["st_k"][:, :, s * ET:(s + 1) * ET], kron[:],
                                  transpose=True)

            def step1(sp, s):
                st = state[sp]
                feaT = st["feaT"]
                if s == 0:
                    st["Tsv"] = tsbp.tile([128, NSL, EW], bf16, tag="Tsv", name=f"Tsv_sb_{sp}")
                    st["Tvs"] = tsbp.tile([128, NSL, EW], bf16, tag="Tvs", name=f"Tvs_sb_{sp}")
                Tsv, Tvs = st["Tsv"], st["Tvs"]
                NH = NSL // 2
                for path, (wmat, T) in enumerate(((wsv, Tsv), (wvs, Tvs))):
                    for h in range(2):
                        Tp = pT.tile([128, NH * ET], f32, tag="T", name=f"T_{sp}_{s}_{path}_{h}")
                        for sl in range(NH):
                            gsl = h * NH + sl
                            nc.tensor.matmul(
                                Tp[:, sl * ET:(sl + 1) * ET],
                                wmat[:, gsl * 128:(gsl + 1) * 128],
                                feaT[:, path, s * ET:(s + 1) * ET],
                                start=True, stop=True)
                        nc.scalar.activation(
                            T[:, h * NH:(h + 1) * NH, s * ET:(s + 1) * ET],
                            Tp[:].rearrange("p (sl e) -> p sl e", e=ET),
                            Copy)

            def stage_B1(sp, half):
                st = state[sp]
                stk, Tsv, Tvs = st["stk"], st["Tsv"], st["Tvs"]
                if 0 in half:
                    st["ms"] = []
                NGD = 4
                for i in half:
                    for jj, T in ((0, Tsv), (1, Tvs)):
                        j = 2 * i + jj
                        m = mmp.tile([128, NSL, EW], bf16, tag=f"m{j}", name=f"m{j}_{sp}")
                        nc.gpsimd.tensor_tensor(
                            m[:, NSL - NGD:NSL, :], T[:, NSL - NGD:NSL, :],
                            stk[:, j, :].unsqueeze(1).broadcast_to([128, NGD, EW]),
                            mul_op)
                        nc.vector.tensor_tensor(
                            m[:, 0:NSL - NGD, :], T[:, 0:NSL - NGD, :],
                            stk[:, j, :].unsqueeze(1).broadcast_to([128, NSL - NGD, EW]),
                            mul_op)
                        st["ms"].append(m)

            def stage_B2(sp, s):
                st = state[sp]
                if s == 0:
                    st["acc"] = pacc.tile([S + V, 2 * EW], f32, tag="acc", name=f"acc_{sp}")
                acc_ss = st["acc"][:, 0:EW]
                st_k = st["st_k"]
                for c in range(NCH):
                    nc.tensor.matmul(
                        acc_ss[:, s * ET:(s + 1) * ET],
                        wssvv[:, c * (S + V):(c + 1) * (S + V)],
                        st_k[:, c, s * ET:(s + 1) * ET],
                        start=(c == 0), stop=(c == NCH - 1))

            def stage_B3(sp):
                st = state[sp]
                acc_v = st["acc"][:, EW:2 * EW]
                for i in range(3):
                    for jj in range(2):
                        m = st["ms"][2 * i + jj]
                        for sl in range(NSL):
                            nc.tensor.matmul(
                                acc_v[i * V:(i + 1) * V, :],
                                sel[:, sl * V:(sl + 1) * V],
                                m[:, sl, :],
                                start=(jj == 0 and sl == 0),
                                stop=(jj == 1 and sl == NSL - 1))

            def stage_C(sp):
                st = state.pop(sp)
                g0 = sp * EW
                acc = st["acc"]
                acc_v = acc[:, EW:2 * EW]
                wgt_sc, wgt_v = st["wgt_sc"], st["wgt_v"]
                sg_sc = postp.tile([S, EW], bf16, tag="sg_sc", name=f"sg_sc_{sp}")
                nc.scalar.activation(sg_sc[:], acc[0:S, 0:EW], Sigm)
                sg_g = postp.tile([V, EW], bf16, tag="sg_g", name=f"sg_g_{sp}")
                nc.scalar.activation(sg_g[:], acc[S:S + V, 0:EW], Sigm)

                sgw = postp.tile([S, EW], bf16, tag="sgw", name=f"sgw_{sp}")
                nc.vector.tensor_tensor(sgw[:], sg_sc[:], wgt_sc[:], mul_op)
                gwv = postp.tile([V, EW], bf16, tag="gwv", name=f"gwv_{sp}")
                nc.vector.tensor_tensor(gwv[:], sg_g[:], wgt_v[:], mul_op)
                gwrep_p = pmlp.tile([3 * V, EW], f32, tag="mlp", name=f"gwrep_{sp}")
                nc.tensor.matmul(gwrep_p[:], sel3, gwv[:], start=True, stop=True)
                gwrep = postp.tile([3 * V, EW], bf16, tag="gwrep", name=f"gwrep_sb_{sp}")
                nc.scalar.activation(gwrep[:], gwrep_p[:], Copy)

                accv_sb = postp.tile([3 * V, EW], bf16, tag="accv_sb", name=f"accv_sb_{sp}")
                nc.scalar.activation(accv_sb[:], acc_v[0:3 * V, :], Copy)
                accs_sb = postp.tile([S, EW], bf16, tag="accs_sb", name=f"accs_sb_{sp}")
                nc.scalar.activation(accs_sb[:], acc[0:S, 0:EW], Copy)
                osc = postp.tile([S, EW], bf16, tag="osc", name=f"osc_{sp}")
                nc.gpsimd.tensor_tensor(osc[:], accs_sb[:], sgw[:], mul_op)
                ovec = postp.tile([3 * V, EW], bf16, tag="ovec", name=f"ovec_{sp}")
                nc.vector.tensor_tensor(ovec[:], accv_sb[:], gwrep[:], mul_op)

                nc.sync.dma_start(d_osc[:, g0:g0 + EW], osc[:])
                nc.sync.dma_start(d_ovec[:, g0:g0 + EW], ovec[:])

            def due(base, r):
                # emit leg for sp where max(0, sp-base) == r
                if r == 0:
                    return [sp for sp in range(0, min(base + 1, NSUP))]
                sp = r + base
                return [sp] if sp < NSUP else []

            for sp in due(2, 0):
                load_fwT(sp)
            for r in range(NSUP + 2):
                if r < NSUP:
                    loads(r)
                for sp in due(2, r + 1):
                    load_fwT(sp)
                if 1 <= r <= NSUP:
                    stage_B2(r - 1, 1)  # s1 chunk matmuls: ktrans-s1 just landed
                    stage_B1(r - 1, (0, 1))  # mults for i=0,1 (ready at round start)
                if r < NSUP:
                    prebuilds(r)
                for sp in due(2, r):
                    mlp1(sp)
                if 1 <= r <= NSUP:
                    stage_B1(r - 1, (2,))  # mults for i=2
                if r < NSUP:
                    krons(r, 0)
                    step1(r, 0)
                    krons(r, 1)
                    step1(r, 1)
                if r >= 2:
                    stage_C(r - 2)
                if 1 <= r <= NSUP:
                    stage_B3(r - 1)   # PE reduces after this round's step1
                if r < NSUP:
                    stage_B2(r, 0)    # s0 chunk matmuls late this round
                for sp in due(1, r):
                    mlp1g(sp)
                    mlp2(sp)
                for sp in due(0, r):
                    mlp2g(sp)
                    mlp3(sp)

    nc.compile()
    return nc


_CACHED = {}


def kernel(fea_in1, fea_in2, fea_weight,
           w_ss_s, w_vv_s, w_ss_g, w_vv_g, w_sv_v, w_vs_v,
           fc_w1, fc_b1, fc_w2, fc_b2, fc_w3, fc_b3, batch_edge):
    fea_in1 = np.asarray(fea_in1, dtype=np.float32)
    fea_in2 = np.asarray(fea_in2, dtype=np.float32)
    fea_weight = np.asarray(fea_weight, dtype=np.float32)

    wd = _prep_weights(np.asarray(w_ss_s, np.float32), np.asarray(w_vv_s, np.float32),
                       np.asarray(w_ss_g, np.float32), np.asarray(w_vv_g, np.float32),
                       np.asarray(w_sv_v, np.float32), np.asarray(w_vs_v, np.float32),
                       np.asarray(fc_w1, np.float32), np.asarray(fc_b1, np.float32),
                       np.asarray(fc_w2, np.float32), np.asarray(fc_b2, np.float32),
                       np.asarray(fc_w3, np.float32), np.asarray(fc_b3, np.float32))

    if "nc" not in _CACHED:
        _CACHED["nc"] = _build_program()
    nc = _CACHED["nc"]

    in_maps = []
    for c in range(NCORES):
        s0 = c * EC
        f1 = np.zeros((EPAD, 160), BF16)
        f1[:EC] = fea_in1[s0:s0 + EC].astype(BF16)
        f2 = np.zeros((EPAD, 160), BF16)
        f2[:EC] = fea_in2[s0:s0 + EC].astype(BF16)
        fea = np.concatenate([f1, f2], axis=1)  # [EPAD, 320]

        feaT = np.zeros((NSUP, S, 2, EW), BF16)
        feaT[:, :, 0, :] = f1[:, :S].reshape(NSUP, EW, S).transpose(0, 2, 1)
        feaT[:, :, 1, :] = f2[:, :S].reshape(NSUP, EW, S).transpose(0, 2, 1)

        # stacks: [NSUP, p=(dw,v), j, e]; j=2i -> x2v[e, v=p%32, i]; j=2i+1 -> x1v
        x1v = f1[:, S:].reshape(EPAD, V, 3)
        x2v = f2[:, S:].reshape(EPAD, V, 3)
        stk = np.empty((NSUP, 128, 6, EW), BF16)
        for i in range(3):
            s2 = x2v[:, :, i].T.reshape(1, V, NSUP, EW)
            s1 = x1v[:, :, i].T.reshape(1, V, NSUP, EW)
            stk[:, :, 2 * i, :] = np.broadcast_to(s2, (4, V, NSUP, EW)) \
                .reshape(128, NSUP, EW).transpose(1, 0, 2)
            stk[:, :, 2 * i + 1, :] = np.broadcast_to(s1, (4, V, NSUP, EW)) \
                .reshape(128, NSUP, EW).transpose(1, 0, 2)

        fwT = np.zeros((FC_IN, EPAD), BF16)
        fwT[:, :EC] = fea_weight[s0:s0 + EC].T.astype(BF16)
        m = {"fea": fea, "feaT": feaT, "stk": stk, "fwT": fwT}
        m.update(wd)
        in_maps.append(m)

    import os
    trace = bool(int(os.environ.get("KERNEL_TRACE", "0")))
    res = run_bass_kernel_spmd(nc, in_maps, core_ids=list(range(NCORES)), trace=trace)
    _CACHED["exec_time_ns"] = res.exec_time_ns

    out = np.empty((E, S + 3 * V), np.float32)
    # vec partition p = i*32+w  ->  output column 64 + 3*w + i
    vec_cols = np.empty(3 * V, np.int64)
    for i in range(3):
        for w in range(V):
            vec_cols[i * V + w] = S + 3 * w + i
    for c in range(NCORES):
        s0 = c * EC
        osc = np.asarray(res.results[c]["out_sc"], dtype=np.float32)[:, :EC]
        ovec = np.asarray(res.results[c]["out_vec"], dtype=np.float32)[:, :EC]
        out[s0:s0 + EC, :S] = osc.T
        out[s0:s0 + EC, vec_cols] = ovec.T
    return out


if __name__ == "__main__":
    rng = np.random.default_rng(0)
    ins = {
        "fea_in1": rng.standard_normal((E, 160)).astype(np.float32),
        "fea_in2": rng.standard_normal((E, 160)).astype(np.float32),
        "fea_weight": rng.standard_normal((E, FC_IN)).astype(np.float32),
        "w_ss_s": rng.standard_normal((S, S, S)).astype(np.float32),
        "w_vv_s": rng.standard_normal((V, V, S)).astype(np.float32),
        "w_ss_g": rng.standard_normal((S, S, V)).astype(np.float32),
        "w_vv_g": rng.standard_normal((V, V, V)).astype(np.float32),
        "w_sv_v": rng.standard_normal((S, V, V)).astype(np.float32),
        "w_vs_v": rng.standard_normal((V, S, V)).astype(np.float32),
        "fc_w1": rng.standard_normal((FC_IN, HID)).astype(np.float32),
        "fc_b1": np.zeros(HID, np.float32),
        "fc_w2": rng.standard_normal((HID, HID)).astype(np.float32),
        "fc_b2": np.zeros(HID, np.float32),
        "fc_w3": rng.standard_normal((HID, S + V)).astype(np.float32),
        "fc_b3": np.zeros(S + V, np.float32),
        "batch_edge": np.zeros(E, np.int32),
    }
    out = kernel(**ins)
    print("kernel out", out.shape, out.dtype, float(np.abs(out).mean()))



# revision 29
# speedup vs baseline: 1.2674x; 1.2674x over previous
"""Trainium2 Bass kernel for nn_EquiConv (e3nn-style FullyConnectedTensorProduct
+ gate + radial-MLP elementwise conv), data-parallel over edges on 8 cores.

Per core, 256-edge supertiles pipelined three rounds deep:
  - ss/vv tensor-product krons are built DIRECTLY k-major on DVE from
    host-replicated block operands (8u x 16v blocking), so no DMA transpose
    is needed; PE accumulates 40 chunk matmuls (256 moving cols each).
  - sv/vs paths factorized: PE contracts the 64-wide scalar side
    (256-col matmuls into PSUM), ACT evicts to bf16, DVE + GPSIMD (via
    scalar_tensor_tensor, cheaper than tensor_tensor on GPSIMD) multiply by
    the replicated x2v/x1v stacks, PE selector-matmuls reduce over v.
  - ACT uses the silu_and_others table set only: Silu for the MLP + scalar
    gate, Tanh for the vector gate (sigmoid(g) = 0.5 + 0.5*tanh(g/2), the
    halves folded into the host-side weights), Copy/Identity for evictions.
  - B1 stack multiplies split ~25/75 between DVE and GPSIMD, flipped
    DVE-heavy on the last supertile when DVE has no krons left.
  - Inputs arrive as one combined DMA per supertile (+1 for the stacks) to
    keep the SP DMA-issue queue off the critical path; outputs leave as one
    [96, 2, EW] tile per supertile.
"""

import sys

sys.path.insert(0, "/opt/trn_rl_repo")

import numpy as np
import ml_dtypes

import concourse.bass as bass
import concourse.bacc as bacc
import concourse.mybir as mybir
import concourse.tile as tile
from concourse.bass_utils import run_bass_kernel_spmd

BF16 = ml_dtypes.bfloat16

E = 20000
S = 64
V = 32
FC_IN = 128
HID = 64
INV_SQRT3 = 0.5773502691896258

NCORES = 8
EC = E // NCORES  # 2500 edges per core
ET = 128
NT = (EC + ET - 1) // ET  # 20 subtiles
EPAD = NT * ET  # 2560
NSUB = 2
EW = NSUB * ET  # 256 edges per supertile
NSUP = EPAD // EW  # 10

A_SC = float(1.0 / np.sqrt(np.float32(S * S + V * V)))
A_VEC = float(1.0 / np.sqrt(np.float32(2 * S * V)))

f32 = mybir.dt.float32
bf16 = mybir.dt.bfloat16

# ss kron blocking: partition p = du*16 + dv (du in 8, dv in 16)
# chunk c = cu*4 + cv; u = cu*8 + du, v = cv*16 + dv
CU_SS, CV_SS = 8, 4    # 32 ss chunks
CU_VV, CV_VV = 4, 2    # 8 vv chunks per group
N_SS = CU_SS * CV_SS
N_VV = CU_VV * CV_VV
NCH = N_SS + 2 * N_VV  # 48 chunks (vv i0+i1 group | vv i2 group)
NSL = 8                # (dw,v) slices per sv/vs step-1
NH = NSL // 2

# combined input tile columns (bf16 [128, CCOMB, EW] per supertile)
C_X1R = 0                  # 8 cols: x1rep
C_X2S = C_X1R + CU_SS      # 4 cols: x2side
C_X1V = C_X2S + CV_SS      # 12 cols: x1vrep (i-major)
C_X2V = C_X1V + 3 * CU_VV  # 6 cols: x2vside (i-major)
C_FEAT = C_X2V + 3 * CV_VV  # 2 cols: x1sT|x2sT (partitions 0:64)
C_FWT = C_FEAT + 2         # 1 col: fwT
CCOMB = C_FWT + 1          # 33

# packed-constant column offsets (bf16 [128, WCONST])
OFF_WSSVV = 0
OFF_WSV = OFF_WSSVV + NCH * (S + V)
OFF_WVS = OFF_WSV + NSL * 4 * V
OFF_SEL = OFF_WVS + NSL * 4 * V
OFF_FC1 = OFF_SEL + NSL * V
OFF_FC2 = OFF_FC1 + HID
OFF_FC3 = OFF_FC2 + HID
OFF_SEL3 = OFF_FC3 + S + V
WCONST = OFF_SEL3 + 3 * V

NDS = (3, 2)      # steady-state DVE slices per path (pool gets NSL-NDS)
NDS_LAST = (6, 6)  # last supertile: DVE-heavy


def _prep_weights(w_ss_s, w_vv_s, w_ss_g, w_vv_g, w_sv_v, w_vs_v,
                  fc_w1, fc_b1, fc_w2, fc_b2, fc_w3, fc_b3):
    """Host-side rearrangement of the shared weights."""
    # gate columns pre-halved: sigmoid(g) = 0.5 + 0.5*tanh(g/2)
    wss = np.concatenate([w_ss_s, 0.5 * w_ss_g], axis=2) * A_SC  # [64,64,96]
    wvv = np.concatenate([w_vv_s, 0.5 * w_vv_g], axis=2) * (A_SC * INV_SQRT3)

    w_ssvv = np.zeros((128, NCH, S + V), np.float32)
    for cu in range(CU_SS):
        for cv in range(CV_SS):
            c = cu * CV_SS + cv
            w_ssvv[:, c, :] = wss[cu * 8:(cu + 1) * 8,
                                  cv * 16:(cv + 1) * 16, :].reshape(128, S + V)
    for g in range(2):
        for cu in range(CU_VV):
            for cv in range(CV_VV):
                c = N_SS + g * N_VV + cu * CV_VV + cv
                w_ssvv[:, c, :] = wvv[cu * 8:(cu + 1) * 8,
                                      cv * 16:(cv + 1) * 16, :].reshape(128, S + V)
    w_ssvv = w_ssvv.reshape(128, NCH * (S + V))

    wsv_mat = (w_sv_v * A_VEC).transpose(0, 2, 1).reshape(S, NSL * 4 * V)
    wvs_mat = (w_vs_v * A_VEC).transpose(1, 2, 0).reshape(S, NSL * 4 * V)

    # selector for the v-reduce: sel[p=(dw,v), s, w'] = 1 iff w' == s*4 + p//32
    sel = np.zeros((128, NSL, V), dtype=np.float32)
    for p in range(128):
        dw = p // 32
        for s in range(NSL):
            sel[p, s, s * 4 + dw] = 1.0

    # gv replication selector, housed at partitions 64:96 (gv lives there)
    sel3 = np.zeros((V, 3 * V), dtype=np.float32)
    for i in range(3):
        for w in range(V):
            sel3[w, i * V + w] = 1.0

    # fc_w3 vec-part pre-halved (second half of the sigmoid->tanh fold)
    fc_w3 = fc_w3.copy()
    fc_w3[:, S:] *= 0.5
    fc_b3 = fc_b3.copy()
    fc_b3[S:] *= 0.5

    wpack = np.zeros((128, WCONST), BF16)
    wpack[:, OFF_WSSVV:OFF_WSSVV + NCH * (S + V)] = w_ssvv.astype(BF16)
    wpack[0:S, OFF_WSV:OFF_WSV + NSL * 4 * V] = wsv_mat.astype(BF16)
    wpack[0:S, OFF_WVS:OFF_WVS + NSL * 4 * V] = wvs_mat.astype(BF16)
    wpack[:, OFF_SEL:OFF_SEL + NSL * V] = sel.reshape(128, NSL * V).astype(BF16)
    wpack[0:FC_IN, OFF_FC1:OFF_FC1 + HID] = fc_w1.astype(BF16)
    wpack[0:HID, OFF_FC2:OFF_FC2 + HID] = fc_w2.astype(BF16)
    wpack[0:HID, OFF_FC3:OFF_FC3 + S + V] = fc_w3.astype(BF16)
    wpack[S:S + V, OFF_SEL3:OFF_SEL3 + 3 * V] = sel3.astype(BF16)
    bpack = np.zeros((S + V, 3), np.float32)
    bpack[0:HID, 0] = fc_b1
    bpack[0:HID, 1] = fc_b2
    bpack[:, 2] = fc_b3
    return {"wpack": wpack, "bpack": bpack}


def _build_program():
    nc = bacc.Bacc("TRN2", target_bir_lowering=False, debug=False)

    d_comb = nc.dram_tensor("comb", [NSUP, 128, CCOMB, EW], bf16,
                            kind="ExternalInput").ap()
    d_stk = nc.dram_tensor("stk", [NSUP, 128, 6, EW], bf16, kind="ExternalInput").ap()
    d_wpack = nc.dram_tensor("wpack", [128, WCONST], bf16, kind="ExternalInput").ap()
    d_bpack = nc.dram_tensor("bpack", [S + V, 3], f32, kind="ExternalInput").ap()

    d_out = nc.dram_tensor("out", [S + V, 2, EPAD], bf16, kind="ExternalOutput").ap()

    Silu = mybir.ActivationFunctionType.Silu
    Tanh = mybir.ActivationFunctionType.Tanh
    Copy = mybir.ActivationFunctionType.Copy
    Ident = mybir.ActivationFunctionType.Identity
    mul_op = mybir.AluOpType.mult
    add_op = mybir.AluOpType.add

    with tile.TileContext(nc) as tc:
        with (
            tc.tile_pool(name="consts", bufs=1) as consts,
            tc.tile_pool(name="io", bufs=2) as io,
            tc.tile_pool(name="iostk", bufs=3) as iostk,
            tc.tile_pool(name="ktr", bufs=2) as ktrp,
            tc.tile_pool(name="tsb", bufs=3) as tsbp,
            tc.tile_pool(name="mm", bufs=2) as mmp,
            tc.tile_pool(name="post", bufs=3) as postp,
            tc.tile_pool(name="pT", bufs=2, space=bass.MemorySpace.PSUM) as pT,
            tc.tile_pool(name="pacc", bufs=2, space=bass.MemorySpace.PSUM) as pacc,
            tc.tile_pool(name="pmlp", bufs=2, space=bass.MemorySpace.PSUM) as pmlp,
        ):
            wpack = consts.tile([128, WCONST], bf16, name="wpack")
            bpack = consts.tile([S + V, 3], f32, name="bpack")
            wssvv = wpack[:, OFF_WSSVV:OFF_WSSVV + NCH * (S + V)]
            wsv = wpack[0:S, OFF_WSV:OFF_WSV + NSL * 4 * V]
            wvs = wpack[0:S, OFF_WVS:OFF_WVS + NSL * 4 * V]
            sel = wpack[:, OFF_SEL:OFF_SEL + NSL * V]
            wfc1 = wpack[0:FC_IN, OFF_FC1:OFF_FC1 + HID]
            wfc2 = wpack[0:HID, OFF_FC2:OFF_FC2 + HID]
            wfc3 = wpack[0:HID, OFF_FC3:OFF_FC3 + S + V]
            sel3 = wpack[S:S + V, OFF_SEL3:OFF_SEL3 + 3 * V]
            bfc1 = bpack[0:HID, 0:1]
            bfc2 = bpack[0:HID, 1:2]
            bfc3 = bpack[0:S + V, 2:3]

            state = {}

            def load_comb(sp):
                st = state.setdefault(sp, {})
                comb = io.tile([128, CCOMB, EW], bf16, tag="comb", name=f"comb_{sp}")
                nc.sync.dma_start(comb[:], d_comb[sp])
                st["comb"] = comb

            def load_stk(sp):
                st = state.setdefault(sp, {})
                stk = iostk.tile([128, 6, EW], bf16, tag="stk", name=f"stk_{sp}")
                nc.sync.dma_start(stk[:], d_stk[sp])
                st["stk"] = stk

            def kron_alloc(sp):
                st = state[sp]
                st["st_k"] = ktrp.tile([128, NCH, EW], bf16, tag="st_k",
                                       name=f"st_k_{sp}")
                st["pv"] = ktrp.tile([128, CU_VV, CV_VV, EW], bf16, tag="pv",
                                     name=f"pv_{sp}")

            def _vv_ins(comb, i):
                in0 = comb[:, C_X1V + i * CU_VV:C_X1V + (i + 1) * CU_VV, :] \
                    .unsqueeze(2).broadcast_to([128, CU_VV, CV_VV, EW])
                in1 = comb[:, C_X2V + i * CV_VV:C_X2V + (i + 1) * CV_VV, :] \
                    .unsqueeze(1).broadcast_to([128, CU_VV, CV_VV, EW])
                return in0, in1

            def kron_pool(sp):
                # round-0 assist only: Pool TT on half the i=1 multiply
                st = state[sp]
                comb = st["comb"]
                if sp != 0:
                    return
                for cu in range(2):
                    nc.gpsimd.tensor_tensor(
                        st["pv"][:, cu, :, :],
                        comb[:, C_X1V + CU_VV + cu, :].unsqueeze(1)
                            .broadcast_to([128, CV_VV, EW]),
                        comb[:, C_X2V + CV_VV:C_X2V + 2 * CV_VV, :],
                        mul_op)

            def krons(sp):
                st = state[sp]
                comb = st["comb"]
                st_k = st["st_k"]
                pv = st["pv"]
                nc.vector.tensor_tensor(
                    st_k[:, 0:N_SS, :].rearrange("p (cu cv) e -> p cu cv e", cv=CV_SS),
                    comb[:, C_X1R:C_X1R + CU_SS, :].unsqueeze(2)
                        .broadcast_to([128, CU_SS, CV_SS, EW]),
                    comb[:, C_X2S:C_X2S + CV_SS, :].unsqueeze(1)
                        .broadcast_to([128, CU_SS, CV_SS, EW]),
                    mul_op)
                kvvA = st_k[:, N_SS:N_SS + N_VV, :] \
                    .rearrange("p (cu cv) e -> p cu cv e", cv=CV_VV)
                kvvB = st_k[:, N_SS + N_VV:NCH, :] \
                    .rearrange("p (cu cv) e -> p cu cv e", cv=CV_VV)
                in0, in1 = _vv_ins(comb, 0)
                nc.vector.tensor_tensor(kvvA, in0, in1, mul_op)
                in0, in1 = _vv_ins(comb, 1)
                if sp == 0:
                    nc.vector.tensor_tensor(pv[:, 2:CU_VV], in0[:, 2:CU_VV],
                                            in1[:, 2:CU_VV], mul_op)
                else:
                    nc.vector.tensor_tensor(pv[:], in0, in1, mul_op)
                in0, in1 = _vv_ins(comb, 2)
                nc.vector.tensor_tensor(kvvB, in0, in1, mul_op)
                nc.vector.tensor_tensor(kvvA, kvvA, pv[:], add_op)

            def step1(sp):
                st = state[sp]
                comb = st["comb"]
                Tsv = tsbp.tile([128, NSL, EW], bf16, tag="Tsv", name=f"Tsv_{sp}")
                Tvs = tsbp.tile([128, NSL, EW], bf16, tag="Tvs", name=f"Tvs_{sp}")
                st["Tsv"], st["Tvs"] = Tsv, Tvs
                for path, (wmat, T) in enumerate(((wsv, Tsv), (wvs, Tvs))):
                    feaTp = comb[0:S, C_FEAT + path, :]
                    for h in range(2):
                        Tp = pT.tile([128, NH, EW], f32, tag="T",
                                     name=f"T_{sp}_{path}_{h}")
                        for sl in range(NH):
                            gsl = h * NH + sl
                            nc.tensor.matmul(
                                Tp[:, sl, :],
                                wmat[:, gsl * 128:(gsl + 1) * 128],
                                feaTp,
                                start=True, stop=True)
                        nc.scalar.activation(T[:, h * NH:(h + 1) * NH, :], Tp[:], Copy)

            def B1_alloc(sp):
                st = state[sp]
                st["ms"] = [
                    mmp.tile([128, 3, NSL, EW], bf16, tag=f"mp{path}",
                             name=f"mp{path}_{sp}")
                    for path in range(2)
                ]

            def B1_dve_i(sp, i_list):
                st = state[sp]
                stk = st["stk"]
                for i in i_list:
                    for path, T in ((0, st["Tsv"]), (1, st["Tvs"])):
                        nc.vector.tensor_tensor(
                            st["ms"][path][:, i, :, :], T[:],
                            stk[:, 2 * i + path, :].unsqueeze(1)
                                .broadcast_to([128, NSL, EW]),
                            mul_op)

            def B1_dve_part(sp, i, path, sl0, sl1):
                st = state[sp]
                stk = st["stk"]
                T = st["Tsv"] if path == 0 else st["Tvs"]
                nc.vector.tensor_tensor(
                    st["ms"][path][:, i, sl0:sl1, :], T[:, sl0:sl1, :],
                    stk[:, 2 * i + path, :].unsqueeze(1)
                        .broadcast_to([128, sl1 - sl0, EW]),
                    mul_op)

            def B1_pool_part(sp, i, path, sl0, sl1):
                st = state[sp]
                stk = st["stk"]
                T = st["Tsv"] if path == 0 else st["Tvs"]
                nc.gpsimd.tensor_tensor(
                    st["ms"][path][:, i, sl0:sl1, :], T[:, sl0:sl1, :],
                    stk[:, 2 * i + path, :].unsqueeze(1)
                        .broadcast_to([128, sl1 - sl0, EW]),
                    mul_op)

            def B2(sp):
                st = state[sp]
                acc = pacc.tile([S + V, 2, EW], f32, tag="acc", name=f"acc_{sp}")
                st["acc"] = acc
                st_k = st["st_k"]
                for c in range(NCH):
                    nc.tensor.matmul(
                        acc[:, 0, :],
                        wssvv[:, c * (S + V):(c + 1) * (S + V)],
                        st_k[:, c, :],
                        start=(c == 0), stop=(c == NCH - 1))

            def B3_i(sp, i):
                st = state[sp]
                acc = st["acc"]
                for path in range(2):
                    m = st["ms"][path]
                    for sl in range(NSL):
                        nc.tensor.matmul(
                            acc[i * V:(i + 1) * V, 1, :],
                            sel[:, sl * V:(sl + 1) * V],
                            m[:, i, sl, :],
                            start=(path == 0 and sl == 0),
                            stop=(path == 1 and sl == NSL - 1))

            def mlp1(sp):
                st = state[sp]
                h1p = pmlp.tile([S + V, EW], f32, tag="mlp", name=f"h1p_{sp}")
                nc.tensor.matmul(h1p[0:HID, :], wfc1, st["comb"][:, C_FWT, :],
                                 start=True, stop=True)
                h1 = postp.tile([HID, EW], bf16, tag="h1", name=f"h1_{sp}")
                nc.scalar.activation(h1[:], h1p[0:HID, :], Silu, bias=bfc1)
                st["h1"] = h1

            def mlp2(sp):
                st = state[sp]
                h2p = pmlp.tile([S + V, EW], f32, tag="mlp", name=f"h2p_{sp}")
                nc.tensor.matmul(h2p[0:HID, :], wfc2, st["h1"][:], start=True, stop=True)
                h2 = postp.tile([HID, EW], bf16, tag="h2", name=f"h2_{sp}")
                nc.scalar.activation(h2[:], h2p[0:HID, :], Silu, bias=bfc2)
                st["h2"] = h2

            def mlp3(sp):
                st = state[sp]
                wp = pmlp.tile([S + V, EW], f32, tag="mlp", name=f"wp_{sp}")
                nc.tensor.matmul(wp[:], wfc3, st["h2"][:], start=True, stop=True)
                wgt = postp.tile([S + V, EW], bf16, tag="wgt", name=f"wgt_{sp}")
                nc.scalar.activation(wgt[:], wp[:], Ident, bias=bfc3)
                st["wgt"] = wgt

            def C_act(sp):
                # sc|g region of acc is final once B2(sp) stops; runs during B3
                st = state[sp]
                acc = st["acc"]
                silu96 = postp.tile([S + V, EW], bf16, tag="silu96", name=f"silu96_{sp}")
                nc.scalar.activation(silu96[:], acc[:, 0, :], Silu)
                tanhg = postp.tile([S + V, EW], bf16, tag="tanhg", name=f"tanhg_{sp}")
                nc.scalar.activation(tanhg[S:S + V, :], acc[S:S + V, 0, :], Tanh)
                st["silu96"], st["tanhg"] = silu96, tanhg

            def C_mid(sp):
                st = state[sp]
                wgt = st["wgt"]
                gv = postp.tile([S + V, EW], bf16, tag="gv", name=f"gv_{sp}")
                nc.vector.scalar_tensor_tensor(
                    gv[S:S + V, :], st["tanhg"][S:S + V, :], 1.0,
                    wgt[S:S + V, :], add_op, mul_op)
                st["gv"] = gv
                otile = postp.tile([S + V, 2, EW], bf16, tag="otile", name=f"otile_{sp}")
                nc.gpsimd.tensor_tensor(otile[:, 1, :], st["silu96"][:], wgt[:],
                                        mul_op)
                st["otile"] = otile

            def C_rep(sp):
                st = state[sp]
                gv = st["gv"]
                gwrep = postp.tile([3 * V, EW], bf16, tag="gwrep", name=f"gwrep_sb_{sp}")
                for i in range(3):
                    nc.scalar.activation(gwrep[i * V:(i + 1) * V, :],
                                         gv[S:S + V, :], Copy)
                st["gwrep"] = gwrep

            def C_out(sp):
                st = state.pop(sp)
                g0 = sp * EW
                otile = st["otile"]
                nc.vector.tensor_tensor(otile[:, 0, :], st["acc"][:, 1, :],
                                        st["gwrep"][:], mul_op)
                nc.sync.dma_start(d_out[:, :, g0:g0 + EW], otile[:])

            for r in range(NSUP + 2):
                # loads / consts
                if r == 0:
                    # PE p-state warm-up: junk matmuls on a zeroed tile so the
                    # tensor engine is at full clock before its first real op
                    warm = consts.tile([128, EW], bf16, name="warm")
                    nc.gpsimd.memset(warm[:], 0.0)
                    warmp = pmlp.tile([S + V, EW], f32, tag="mlp", name="warmp")
                    for _ in range(36):
                        nc.tensor.matmul(warmp[:], warm[:, 0:S + V], warm[:],
                                         start=True, stop=True)
                    comb0 = io.tile([128, CCOMB, EW], bf16, tag="comb", name="comb_0")
                    state.setdefault(0, {})["comb"] = comb0
                    nc.sync.dma_start(comb0[:, C_X1R:C_X1V, :],
                                      d_comb[0][:, C_X1R:C_X1V, :])
                    nc.sync.dma_start(comb0[:, C_X1V:C_FEAT, :],
                                      d_comb[0][:, C_X1V:C_FEAT, :])
                    nc.sync.dma_start(comb0[:, C_FEAT:CCOMB, :],
                                      d_comb[0][:, C_FEAT:CCOMB, :])
                    load_stk(0)
                    nc.sync.dma_start(wpack[:, OFF_WSV:OFF_SEL],
                                      d_wpack[:, OFF_WSV:OFF_SEL])
                    nc.sync.dma_start(wpack[:, OFF_WSSVV:OFF_WSV],
                                      d_wpack[:, OFF_WSSVV:OFF_WSV])
                    nc.sync.dma_start(wpack[:, OFF_SEL:WCONST],
                                      d_wpack[:, OFF_SEL:WCONST])
                    nc.sync.dma_start(bpack[:], d_bpack)
                    comb1 = io.tile([128, CCOMB, EW], bf16, tag="comb", name="comb_1")
                    state.setdefault(1, {})["comb"] = comb1
                    nc.sync.dma_start(comb1[:, C_X1R:C_X1V, :],
                                      d_comb[1][:, C_X1R:C_X1V, :])
                    nc.sync.dma_start(comb1[:, C_X1V:CCOMB, :],
                                      d_comb[1][:, C_X1V:CCOMB, :])
                elif r + 1 < NSUP:
                    load_comb(r + 1)
                if 1 <= r < NSUP:
                    load_stk(r)
                last = (r == NSUP)
                if 1 <= r <= NSUP:
                    B1_alloc(r - 1)
                    if last:
                        B1_dve_i(r - 1, [1, 2])
                        B1_dve_part(r - 1, 0, 0, 0, NSL)
                    else:
                        B1_dve_i(r - 1, [2])
                        B1_dve_part(r - 1, 1, 0, 0, NSL - 1)
                if r < NSUP:
                    kron_alloc(r)
                    kron_pool(r)
                if 1 <= r <= NSUP:
                    if last:
                        B1_pool_part(r - 1, 0, 1, 0, NSL)
                    else:
                        B1_pool_part(r - 1, 1, 0, NSL - 1, NSL)
                        B1_pool_part(r - 1, 1, 1, 0, NSL)
                        B1_pool_part(r - 1, 0, 0, 0, NSL)
                        B1_pool_part(r - 1, 0, 1, 0, NSL)
                if r < NSUP:
                    krons(r)
                if 1 <= r <= NSUP and not last:
                    B2(r - 1)
                if 2 <= r <= NSUP:
                    B3_i(r - 2, 0)  # i0 group delayed a round: m long-ready
                if 1 <= r <= NSUP:
                    C_act(r - 1)
                if r < NSUP:
                    mlp1(r)
                if 1 <= r <= NSUP:
                    mlp2(r - 1)
                    mlp3(r - 1)
                    if last:
                        B3_i(r - 1, 1)
                        B3_i(r - 1, 2)
                    else:
                        B3_i(r - 1, 2)
                        B3_i(r - 1, 1)
                if r == 0:
                    step1(0)
                    step1(1)
                elif r + 1 < NSUP:
                    step1(r + 1)
                if r == NSUP - 1:
                    B2(r)  # pre-run the last supertile's chunk matmuls
                if last:
                    B3_i(r - 1, 0)  # tail: no extra round for i0
                if 2 <= r:
                    C_out(r - 2)
                if 1 <= r <= NSUP:
                    C_mid(r - 1)
                    C_rep(r - 1)

    nc.compile()
    return nc


_CACHED = {}


def kernel(fea_in1, fea_in2, fea_weight,
           w_ss_s, w_vv_s, w_ss_g, w_vv_g, w_sv_v, w_vs_v,
           fc_w1, fc_b1, fc_w2, fc_b2, fc_w3, fc_b3, batch_edge):
    fea_in1 = np.asarray(fea_in1, dtype=np.float32)
    fea_in2 = np.asarray(fea_in2, dtype=np.float32)
    fea_weight = np.asarray(fea_weight, dtype=np.float32)

    wd = _prep_weights(np.asarray(w_ss_s, np.float32), np.asarray(w_vv_s, np.float32),
                       np.asarray(w_ss_g, np.float32), np.asarray(w_vv_g, np.float32),
                       np.asarray(w_sv_v, np.float32), np.asarray(w_vs_v, np.float32),
                       np.asarray(fc_w1, np.float32), np.asarray(fc_b1, np.float32),
                       np.asarray(fc_w2, np.float32), np.asarray(fc_b2, np.float32),
                       np.asarray(fc_w3, np.float32), np.asarray(fc_b3, np.float32))

    if "nc" not in _CACHED:
        _CACHED["nc"] = _build_program()
    nc = _CACHED["nc"]

    in_maps = []
    for c in range(NCORES):
        s0 = c * EC
        f1 = np.zeros((EPAD, 160), BF16)
        f1[:EC] = fea_in1[s0:s0 + EC].astype(BF16)
        f2 = np.zeros((EPAD, 160), BF16)
        f2[:EC] = fea_in2[s0:s0 + EC].astype(BF16)
        x1s, x1v = f1[:, :S], f1[:, S:].reshape(EPAD, V, 3)
        x2s, x2v = f2[:, :S], f2[:, S:].reshape(EPAD, V, 3)

        comb = np.zeros((NSUP, 128, CCOMB, EW), BF16)
        # x1rep[t, du*16+dv, cu, e] = x1s[t*EW+e, cu*8+du]
        b = x1s.reshape(NSUP, EW, CU_SS, 8).transpose(0, 3, 2, 1)  # [t,du,cu,e]
        comb[:, :, C_X1R:C_X1R + CU_SS, :] = \
            np.broadcast_to(b[:, :, None], (NSUP, 8, 16, CU_SS, EW)) \
            .reshape(NSUP, 128, CU_SS, EW)
        # x2side[t, du*16+dv, cv, e] = x2s[t*EW+e, cv*16+dv]
        b = x2s.reshape(NSUP, EW, CV_SS, 16).transpose(0, 3, 2, 1)  # [t,dv,cv,e]
        comb[:, :, C_X2S:C_X2S + CV_SS, :] = \
            np.broadcast_to(b[:, None], (NSUP, 8, 16, CV_SS, EW)) \
            .reshape(NSUP, 128, CV_SS, EW)
        # x1vrep[t, du*16+dv, i, cu, e] = x1v[t*EW+e, cu*8+du, i]
        b = x1v.reshape(NSUP, EW, CU_VV, 8, 3).transpose(0, 3, 4, 2, 1)  # [t,du,i,cu,e]
        comb[:, :, C_X1V:C_X1V + 3 * CU_VV, :] = \
            np.broadcast_to(b[:, :, None], (NSUP, 8, 16, 3, CU_VV, EW)) \
            .reshape(NSUP, 128, 3 * CU_VV, EW)
        # x2vside[t, du*16+dv, i, cv, e] = x2v[t*EW+e, cv*16+dv, i]
        b = x2v.reshape(NSUP, EW, CV_VV, 16, 3).transpose(0, 3, 4, 2, 1)  # [t,dv,i,cv,e]
        comb[:, :, C_X2V:C_X2V + 3 * CV_VV, :] = \
            np.broadcast_to(b[:, None], (NSUP, 8, 16, 3, CV_VV, EW)) \
            .reshape(NSUP, 128, 3 * CV_VV, EW)
        # feaT
        comb[:, 0:S, C_FEAT, :] = x1s.reshape(NSUP, EW, S).transpose(0, 2, 1)
        comb[:, 0:S, C_FEAT + 1, :] = x2s.reshape(NSUP, EW, S).transpose(0, 2, 1)
        # fwT
        fw = np.zeros((EPAD, FC_IN), BF16)
        fw[:EC] = fea_weight[s0:s0 + EC].astype(BF16)
        comb[:, :, C_FWT, :] = fw.reshape(NSUP, EW, FC_IN).transpose(0, 2, 1)

        # stacks: [NSUP, p=(dw,v), j, e]; j=2i -> x2v[e, v=p%32, i]; j=2i+1 -> x1v
        stk = np.empty((NSUP, 128, 6, EW), BF16)
        for i in range(3):
            s2 = x2v[:, :, i].T.reshape(1, V, NSUP, EW)
            s1 = x1v[:, :, i].T.reshape(1, V, NSUP, EW)
            stk[:, :, 2 * i, :] = np.broadcast_to(s2, (4, V, NSUP, EW)) \
                .reshape(128, NSUP, EW).transpose(1, 0, 2)
            stk[:, :, 2 * i + 1, :] = np.broadcast_to(s1, (4, V, NSUP, EW)) \
                .reshape(128, NSUP, EW).transpose(1, 0, 2)

        m = {"comb": comb, "stk": stk}
        m.update(wd)
        in_maps.append(m)

    import os
    trace = bool(int(os.environ.get("KERNEL_TRACE", "0")))
    res = run_bass_kernel_spmd(nc, in_maps, core_ids=list(range(NCORES)), trace=trace)
    _CACHED["exec_time_ns"] = res.exec_time_ns

    out = np.empty((E, S + 3 * V), np.float32)
    # vec partition p = i*32+w  ->  output column 64 + 3*w + i
    vec_cols = np.empty(3 * V, np.int64)
    for i in range(3):
        for w in range(V):
            vec_cols[i * V + w] = S + 3 * w + i
    for c in range(NCORES):
        s0 = c * EC
        o = np.asarray(res.results[c]["out"], dtype=np.float32)
        out[s0:s0 + EC, :S] = o[0:S, 1, :EC].T
        out[s0:s0 + EC, vec_cols] = o[:, 0, :EC].T
    return out


if __name__ == "__main__":
    rng = np.random.default_rng(0)
    ins = {
        "fea_in1": rng.standard_normal((E, 160)).astype(np.float32),
        "fea_in2": rng.standard_normal((E, 160)).astype(np.float32),
        "fea_weight": rng.standard_normal((E, FC_IN)).astype(np.float32),
        "w_ss_s": rng.standard_normal((S, S, S)).astype(np.float32),
        "w_vv_s": rng.standard_normal((V, V, S)).astype(np.float32),
        "w_ss_g": rng.standard_normal((S, S, V)).astype(np.float32),
        "w_vv_g": rng.standard_normal((V, V, V)).astype(np.float32),
        "w_sv_v": rng.standard_normal((S, V, V)).astype(np.float32),
        "w_vs_v": rng.standard_normal((V, S, V)).astype(np.float32),
        "fc_w1": rng.standard_normal((FC_IN, HID)).astype(np.float32),
        "fc_b1": np.zeros(HID, np.float32),
        "fc_w2": rng.standard_normal((HID, HID)).astype(np.float32),
        "fc_b2": np.zeros(HID, np.float32),
        "fc_w3": rng.standard_normal((HID, S + V)).astype(np.float32),
        "fc_b3": np.zeros(S + V, np.float32),
        "batch_edge": np.zeros(E, np.int32),
    }
    out = kernel(**ins)
    print("kernel out", out.shape, out.dtype, float(np.abs(out).mean()))
